# revision 10
# baseline (speedup 1.0000x reference)
"""Bass/Trainium2 kernel for a 12-head self-attention block
(B=8, T=1024, C=768), data-parallel across 8 NeuronCores (one batch
element per core).

Per-core computation (batch element b):
  qkv   = x @ W_attn + b_attn            [T, 3C]
  scoresT[k, q] = k_h . q_h / 8 (+ mask bias), keys on partitions
  e     = exp(scoresT) (unnormalized; denominator accumulated via a
          ones-column appended to v in the AV matmul)
  out_h = (v_ext.T @ e) / denom-row
  y     = concat(out_h) @ W_proj + b_proj

v2 changes vs the first working version:
  - all matmul operands are bf16 (same 1 cycle/row PE rate as fp32r,
    but half the HBM/SBUF traffic; fp32 PSUM accumulation throughout)
  - host pre-lays-out every weight so each DMA is 128 partitions of
    large contiguous runs (no 4-byte or 256-byte packet storms)
  - bias seeding matmuls removed: host pre-broadcasts b_attn/b_proj to
    [128, C] and the PSUM->SBUF copy becomes a DVE add
  - the softmax denominator partition-broadcast uses one small
    SBUF->SBUF DMA (partition 64 -> 0) + gpsimd.partition_broadcast
    instead of a DRAM round trip
  - v_ext's ones column comes from a DVE broadcast copy, not DMA
  - emission order: qk chunks for pair 0 and pair-0 q-block-0 scores
    go first so ScalarE exp (the secondary bottleneck) starts during
    the v phase; W_proj loads last

Layout scheme (no on-device transposes anywhere):
  - host passes xT = x[b].T                       [C, T]
  - qT/kT computed as  qkT[c', t] = W_attn[:, :1536].T @ x.T
  - v computed as       v[t, c'] = x @ W_attn[:, 1536:]
  - scoresT[k, q] = kT_h.T @ qT_h; head pairs (2j, 2j+1) sit at
    partition offsets 0/64, so their score matmuls run packed on
    disjoint PE row groups, sharing one [128, 1024] PSUM tile and a
    single Exp ACTIVATE
  - AV: out_extT[d_ext, q] = v_ext.T @ expT, v_ext = [v_h | 1];
    row 64 of the 65-row result is the softmax denominator
  - projection: y[t, c2] = concatT.T @ W_proj
"""

import sys

if "/opt/trn_rl_repo" not in sys.path:
    sys.path.insert(0, "/opt/trn_rl_repo")

from contextlib import ExitStack

import ml_dtypes
import numpy as np

import concourse.bass as bass
import concourse.tile as tile
from concourse import bacc, mybir
from concourse import bass_utils

N_HEAD = 12
B = 8
T = 1024
C = 768
HD = 64
KO = C // 128          # 6 contraction chunks of 128
TC = T // 128          # 8 token chunks of 128
QN = T // 512          # 2 query chunks of 512
NPAIR = N_HEAD // 2    # 6 head pairs

F32 = mybir.dt.float32
BF16 = mybir.dt.bfloat16
AF = mybir.ActivationFunctionType

_cache: dict = {}


def _emit_kernel(tc_ctx, aps):
    nc = tc_ctx.nc
    ctx = aps["ctx"]
    xT_d, wqk_d, wv_d, wp_d, bqk_d, bv_d, bp_d, mb_d, y_d = (
        aps["xT"], aps["Wqk"], aps["Wv"], aps["Wp"], aps["bqk"], aps["bv"],
        aps["bp"], aps["mb"], aps["y"],
    )

    const = ctx.enter_context(tc_ctx.tile_pool(name="const", bufs=1))
    wqk_pool = ctx.enter_context(tc_ctx.tile_pool(name="wqk", bufs=8))
    e_pool = ctx.enter_context(tc_ctx.tile_pool(name="e", bufs=12))
    r_pool = ctx.enter_context(tc_ctx.tile_pool(name="r", bufs=2))
    r0_pool = ctx.enter_context(tc_ctx.tile_pool(name="r0", bufs=2))
    rb_pool = ctx.enter_context(tc_ctx.tile_pool(name="rb", bufs=2))
    tmp_pool = ctx.enter_context(tc_ctx.tile_pool(name="tmp", bufs=3))
    out_pool = ctx.enter_context(tc_ctx.tile_pool(name="out", bufs=2))

    # PSUM: 4 banks of [128,512] accumulators (tag-shared ring across all
    # phases) + 4 banks of [128,1024] score tiles.
    acc_ps = ctx.enter_context(tc_ctx.tile_pool(name="accps", bufs=4, space="PSUM"))
    sc_ps = ctx.enter_context(tc_ctx.tile_pool(name="scps", bufs=2, space="PSUM"))

    # ---- persistent SBUF tensors -------------------------------------
    xT_sb = const.tile([128, KO, T], BF16)
    wv_sb = const.tile([128, KO, C], BF16)       # W_attn[:, 1536:2304]
    wp_sb = const.tile([128, KO, C], BF16)       # W_proj
    # per head-pair j: [:, 0, :] = qT chunk j (later overwritten by the
    # pair's normalized concat output), [:, 1, :] = kT chunk 6+j
    qk_sb = [const.tile([128, 2, T], BF16, name=f"qk_{j}") for j in range(NPAIR)]
    v_sb = const.tile([128, TC, N_HEAD, HD + 1], BF16)  # +1 = ones column
    bqk_sb = const.tile([128, 12], F32)
    mb_sb = const.tile([128, TC], F32)
    bv_sb = const.tile([128, C], F32)
    bp_sb = const.tile([128, C], F32)
    one_sb = const.tile([128, 1], BF16)

    # input DMAs in first-use order: pair-0 qk weights, then x, then wv
    wqk06 = []
    for m in (0, 6):
        w = wqk_pool.tile([128, KO, 128], BF16, tag="wqk", name=f"wqk_{m}")
        nc.sync.dma_start(w[:], wqk_d[m].rearrange("p (ko n) -> p ko n", ko=KO))
        wqk06.append(w)
    xT_r = xT_d.rearrange("(ko p) t -> p ko t", p=128)
    for ko in range(KO):
        nc.sync.dma_start(xT_sb[:, ko], xT_r[:, ko])
    wv_r = wv_d.rearrange("p (ko n) -> p ko n", ko=KO)
    for ko in range(KO):
        nc.sync.dma_start(wv_sb[:, ko], wv_r[:, ko])
    nc.gpsimd.dma_start(bqk_sb[:], bqk_d)
    nc.gpsimd.dma_start(mb_sb[:], mb_d)
    nc.gpsimd.dma_start(bv_sb[:], bv_d)

    # ~4us of throwaway matmuls on a zeroed tile: keeps the PE busy
    # while inputs stream in so HAM promotes to full throughput early
    wu_sb = const.tile([128, 512], BF16)
    nc.vector.memset(wu_sb[:], 0.0)
    wu_ps = acc_ps.tile([128, 512], F32, tag="acc", name="wu_ps")
    for i in range(20):
        nc.tensor.matmul(
            wu_ps, wu_sb[:, 0:128], wu_sb[:], start=True, stop=True,
            skip_group_check=True,
        )

    # ones column of v_ext via DVE (no DMA)
    nc.vector.memset(one_sb[:], 1.0)
    for tcc in range(TC):
        nc.vector.tensor_copy(
            out=v_sb[:, tcc, :, HD],
            in_=one_sb[:, 0:1].to_broadcast((128, N_HEAD)),
        )

    segs = [(0, 512), (512, 256)]

    # ---- phase 1b: v[t, c'] for c' in [1536, 2304) -------------------
    def emit_v_chunk(tcc):
        pss = [
            acc_ps.tile([128, 512], F32, tag="acc", name=f"ps1b_{tcc}_{i}")
            for i in range(2)
        ]
        for ko in range(KO):
            for j, (off, w) in enumerate(segs):
                nc.tensor.matmul(
                    pss[j][:, :w],
                    xT_sb[:, ko, tcc * 128 : (tcc + 1) * 128],
                    wv_sb[:, ko, off : off + w],
                    start=(ko == 0),
                    stop=(ko == KO - 1),
                )
        for j, (off, w) in enumerate(segs):
            nc.vector.tensor_tensor(
                v_sb[:, tcc, off // HD : (off + w) // HD, 0:HD],
                pss[j][:, :w].rearrange("p (h d) -> p h d", d=HD),
                bv_sb[:, off : off + w].rearrange("p (h d) -> p h d", d=HD),
                mybir.AluOpType.add,
            )

    # ---- phase 1a (per pair): qkT chunks j and 6+j -------------------
    def emit_qk_chunk(j, half, m, wqk=None):
        # half 0 -> qT chunk (m = j), half 1 -> kT chunk (m = 6 + j)
        if wqk is None:
            wqk = wqk_pool.tile([128, KO, 128], BF16, tag="wqk", name=f"wqk_{m}")
            nc.sync.dma_start(
                wqk[:], wqk_d[m].rearrange("p (ko n) -> p ko n", ko=KO)
            )
        pss = [
            acc_ps.tile([128, 512], F32, tag="acc", name=f"ps1a_{m}_{i}")
            for i in range(QN)
        ]
        for ko in range(KO):
            for nq in range(QN):
                nc.tensor.matmul(
                    pss[nq],
                    wqk[:, ko, :],
                    xT_sb[:, ko, nq * 512 : (nq + 1) * 512],
                    start=(ko == 0),
                    stop=(ko == KO - 1),
                )
        # psum -> SBUF with per-partition bias add (b_attn) on DVE
        for nq in range(QN):
            nc.vector.tensor_tensor(
                qk_sb[j][:, half, nq * 512 : (nq + 1) * 512],
                pss[nq],
                bqk_sb[:, m : m + 1].to_broadcast((128, 512)),
                mybir.AluOpType.add,
            )

    # ---- filler queue: independent PE work (qk/proj chunks) emitted
    # in small slices inside the attention streams' stall points -------
    filler: list = []          # list of [label, generator]

    def pull_filler(n):
        # emit up to n matmul-sized units of filler work
        while n > 0 and filler:
            try:
                next(filler[0][1])
                n -= 1
            except StopIteration:
                filler.pop(0)

    def drain_filler(label):
        # emit everything up to and including the chunk named `label`
        while filler and any(f[0] == label for f in filler):
            try:
                next(filler[0][1])
            except StopIteration:
                filler.pop(0)

    # ---- attention for one head pair, one 512-query block ------------
    def emit_scores_exp(j, qc):
        qk = qk_sb[j]
        qsl = slice(qc * 512, (qc + 1) * 512)
        es = []
        for kc in range(TC):
            ksl = slice(kc * 128, (kc + 1) * 128)
            sc = sc_ps.tile([128, 1024], F32, tag="sc", name=f"sc_{j}_{qc}_{kc}")
            # head a (partitions 0-63) and head b (64-127): disjoint PE
            # row groups -> the two matmuls run packed
            nc.tensor.matmul(
                sc[:, 0:512], qk[0:64, 1, ksl], qk[0:64, 0, qsl],
                start=True, stop=True,
            )
            nc.tensor.matmul(
                sc[:, 512:1024], qk[64:128, 1, ksl], qk[64:128, 0, qsl],
                start=True, stop=True,
            )
            e = e_pool.tile([128, 1024], BF16, tag="e", name=f"e_{j}_{qc}_{kc}")
            nc.scalar.activation(
                e, sc, AF.Exp, bias=mb_sb[:, kc : kc + 1], scale=0.125
            )
            es.append(e)
        return es

    def emit_attention_stream(j, qc):
        # joint scores -> exp -> AV pipeline with the PE-FIFO-friendly
        # ordering: AV for chunk kc-1 is emitted after the score matmuls
        # for chunk kc, plus ~1.5 filler matmuls per round
        qk = qk_sb[j]
        qsl = slice(qc * 512, (qc + 1) * 512)
        ava = acc_ps.tile([65, 512], F32, tag="acc", name=f"ava_{j}_{qc}")
        avb = acc_ps.tile([65, 512], F32, tag="acc", name=f"avb_{j}_{qc}")

        def emit_av(kc, e):
            nc.tensor.matmul(
                ava, v_sb[:, kc, 2 * j, :], e[:, 0:512],
                start=(kc == 0), stop=(kc == TC - 1),
            )
            nc.tensor.matmul(
                avb, v_sb[:, kc, 2 * j + 1, :], e[:, 512:1024],
                start=(kc == 0), stop=(kc == TC - 1),
            )

        prev = None
        for kc in range(TC):
            ksl = slice(kc * 128, (kc + 1) * 128)
            sc = sc_ps.tile([128, 1024], F32, tag="sc", name=f"sc_{j}_{qc}_{kc}")
            nc.tensor.matmul(
                sc[:, 0:512], qk[0:64, 1, ksl], qk[0:64, 0, qsl],
                start=True, stop=True,
            )
            nc.tensor.matmul(
                sc[:, 512:1024], qk[64:128, 1, ksl], qk[64:128, 0, qsl],
                start=True, stop=True,
            )
            e = e_pool.tile([128, 1024], BF16, tag="e", name=f"e_{j}_{qc}_{kc}")
            nc.scalar.activation(
                e, sc, AF.Exp, bias=mb_sb[:, kc : kc + 1], scale=0.125
            )
            if prev is not None:
                emit_av(*prev)
            prev = (kc, e)
            pull_filler(2 if kc % 2 else 1)
        emit_av(*prev)
        emit_norm(j, qc, ava, avb)
        pull_filler(2)

    def emit_av_norm(j, qc, es):
        qk = qk_sb[j]
        ava = acc_ps.tile([65, 512], F32, tag="acc", name=f"ava_{j}_{qc}")
        avb = acc_ps.tile([65, 512], F32, tag="acc", name=f"avb_{j}_{qc}")
        for kc in range(TC):
            nc.tensor.matmul(
                ava, v_sb[:, kc, 2 * j, :], es[kc][:, 0:512],
                start=(kc == 0), stop=(kc == TC - 1),
            )
            nc.tensor.matmul(
                avb, v_sb[:, kc, 2 * j + 1, :], es[kc][:, 512:1024],
                start=(kc == 0), stop=(kc == TC - 1),
            )
        emit_norm(j, qc, ava, avb)

    def emit_norm(j, qc, ava, avb):
        # normalize: denominator rows -> partition 0 via one small
        # SBUF->SBUF DMA -> fast reciprocal -> gpsimd partition
        # broadcast -> multiply into the concat slots
        qk = qk_sb[j]
        qsl = slice(qc * 512, (qc + 1) * 512)
        r_t = r_pool.tile([65, 1024], F32, tag="r", name=f"r_{j}_{qc}")
        nc.vector.tensor_copy(out=r_t[64:65, 0:512], in_=ava[64:65, :])
        nc.vector.tensor_copy(out=r_t[64:65, 512:1024], in_=avb[64:65, :])
        r0 = r0_pool.tile([1, 1024], F32, tag="r0", name=f"r0_{j}_{qc}")
        nc.gpsimd.dma_start(r0[:], r_t[64:65, :])
        rc = r0_pool.tile([1, 1024], F32, tag="rc", name=f"rc_{j}_{qc}")
        nc.vector.reciprocal_approx_fast(out=rc[:], in_=r0[:])
        rb = rb_pool.tile([64, 1024], F32, tag="rb", name=f"rb_{j}_{qc}")
        nc.gpsimd.partition_broadcast(rb[:], rc[:])
        # head a -> concat partitions 0-63 (directly into qT half)
        nc.vector.tensor_mul(
            out=qk[0:64, 0, qsl], in0=ava[0:64, :], in1=rb[:, 0:512]
        )
        # head b -> concat partitions 64-127 (via SBUF->SBUF DMA shift)
        t_sb = tmp_pool.tile([64, 512], BF16, tag="tmp", name=f"tmp_{j}_{qc}")
        nc.vector.tensor_mul(out=t_sb[:], in0=avb[0:64, :], in1=rb[:, 512:1024])
        nc.gpsimd.dma_start(qk[64:128, 0, qsl], t_sb[:])

    # ---- phase 4: one token chunk of y = concatT.T @ W_proj ----------
    def emit_proj_chunk(tcc):
        pss = [
            acc_ps.tile([128, 512], F32, tag="acc", name=f"ps4_{tcc}_{i}")
            for i in range(2)
        ]
        for ko in range(KO):
            for j, (off, w) in enumerate(segs):
                nc.tensor.matmul(
                    pss[j][:, :w],
                    qk_sb[ko][:, 0, tcc * 128 : (tcc + 1) * 128],
                    wp_sb[:, ko, off : off + w],
                    start=(ko == 0),
                    stop=(ko == KO - 1),
                )
        o_sb = out_pool.tile([128, C], F32, tag="out", name=f"o_{tcc}")
        for j, (off, w) in enumerate(segs):
            nc.vector.tensor_tensor(
                o_sb[:, off : off + w],
                pss[j][:, :w],
                bp_sb[:, off : off + w],
                mybir.AluOpType.add,
            )
        nc.sync.dma_start(y_d[tcc * 128 : (tcc + 1) * 128, :], o_sb[:])

    # ---- filler generators -------------------------------------------
    def gen_qk_chunk(j, half, m):
        # DMA issued eagerly at queue time; matmuls stream in units
        wqk = wqk_pool.tile([128, KO, 128], BF16, tag="wqk", name=f"wqk_{m}")
        nc.sync.dma_start(wqk[:], wqk_d[m].rearrange("p (ko n) -> p ko n", ko=KO))

        def gen():
            pss = [
                acc_ps.tile([128, 512], F32, tag="acc", name=f"ps1a_{m}_{i}")
                for i in range(QN)
            ]
            for ko in range(KO):
                for nq in range(QN):
                    nc.tensor.matmul(
                        pss[nq],
                        wqk[:, ko, :],
                        xT_sb[:, ko, nq * 512 : (nq + 1) * 512],
                        start=(ko == 0),
                        stop=(ko == KO - 1),
                    )
                    yield
            for nq in range(QN):
                nc.vector.tensor_tensor(
                    qk_sb[j][:, half, nq * 512 : (nq + 1) * 512],
                    pss[nq],
                    bqk_sb[:, m : m + 1].to_broadcast((128, 512)),
                    mybir.AluOpType.add,
                )

        return gen()

    def gen_av_norm(j, qc, es):
        def gen():
            qk = qk_sb[j]
            ava = acc_ps.tile([65, 512], F32, tag="acc", name=f"ava_{j}_{qc}")
            avb = acc_ps.tile([65, 512], F32, tag="acc", name=f"avb_{j}_{qc}")
            for kc in range(TC):
                nc.tensor.matmul(
                    ava, v_sb[:, kc, 2 * j, :], es[kc][:, 0:512],
                    start=(kc == 0), stop=(kc == TC - 1),
                )
                nc.tensor.matmul(
                    avb, v_sb[:, kc, 2 * j + 1, :], es[kc][:, 512:1024],
                    start=(kc == 0), stop=(kc == TC - 1),
                )
                yield
            emit_norm(j, qc, ava, avb)

        return gen()

    def gen_proj_chunk(tcc):
        def gen():
            pss = [
                acc_ps.tile([128, 512], F32, tag="acc", name=f"ps4_{tcc}_{i}")
                for i in range(2)
            ]
            for ko in range(KO):
                for jj, (off, w) in enumerate(segs):
                    nc.tensor.matmul(
                        pss[jj][:, :w],
                        qk_sb[ko][:, 0, tcc * 128 : (tcc + 1) * 128],
                        wp_sb[:, ko, off : off + w],
                        start=(ko == 0),
                        stop=(ko == KO - 1),
                    )
                    yield
            o_sb = out_pool.tile([128, C], F32, tag="out", name=f"o_{tcc}")
            for jj, (off, w) in enumerate(segs):
                nc.vector.tensor_tensor(
                    o_sb[:, off : off + w],
                    pss[jj][:, :w],
                    bp_sb[:, off : off + w],
                    mybir.AluOpType.add,
                )
            nc.sync.dma_start(y_d[tcc * 128 : (tcc + 1) * 128, :], o_sb[:])

        return gen()

    # ---- schedule ----------------------------------------------------
    # qk chunks for pair 0 + its first score/exp block go first so
    # ScalarE starts its (long) exp stream while the PE does the v phase
    emit_qk_chunk(0, 0, 0, wqk06[0])
    emit_qk_chunk(0, 1, 6, wqk06[1])
    es00 = emit_scores_exp(0, 0)
    for tcc in range(TC):
        emit_v_chunk(tcc)
    emit_qk_chunk(1, 0, 1)
    emit_qk_chunk(1, 1, 7)

    # round one: every pair's first 512-query block. Pair 0's AV (its
    # exps ran during the v phase) and later pairs' qk chunks ride
    # along as filler inside the exp-bound streams. Building the
    # filler queue issues the remaining wqk DMAs eagerly; W_proj's DMA
    # must come after them on the sync queue (it is needed much later).
    filler.append(["av00", gen_av_norm(0, 0, es00)])
    for m in range(2, NPAIR):
        filler.append([f"qk{m}", gen_qk_chunk(m, 0, m)])
        filler.append([f"qk{m}", gen_qk_chunk(m, 1, 6 + m)])
    wp_r = wp_d.rearrange("p (ko n) -> p ko n", ko=KO)
    for ko in range(KO):
        nc.sync.dma_start(wp_sb[:, ko], wp_r[:, ko])
    nc.gpsimd.dma_start(bp_sb[:], bp_d)
    for j in range(1, NPAIR):
        if j >= 2:
            drain_filler(f"qk{j}")
        emit_attention_stream(j, 0)
    drain_filler("av00")

    # round two: second q-block; the first half of the projection
    # (t 0:512, complete after round one) is the filler
    for tcc in range(4):
        filler.append([f"proj{tcc}", gen_proj_chunk(tcc)])
    for j in range(NPAIR):
        emit_attention_stream(j, 1)
    pull_filler(10 ** 9)
    for tcc in range(4, TC):
        emit_proj_chunk(tcc)


def _get_program():
    if "nc" in _cache:
        return _cache["nc"]
    nc = bacc.Bacc(
        "TRN2", target_bir_lowering=False, debug=False, enable_asserts=True
    )
    aps = {
        "xT": nc.dram_tensor("xT", [C, T], BF16, kind="ExternalInput").ap(),
        "Wqk": nc.dram_tensor("Wqk", [12, 128, C], BF16, kind="ExternalInput").ap(),
        "Wv": nc.dram_tensor("Wv", [128, KO * C], BF16, kind="ExternalInput").ap(),
        "Wp": nc.dram_tensor("Wp", [128, KO * C], BF16, kind="ExternalInput").ap(),
        "bqk": nc.dram_tensor("bqk", [128, 12], F32, kind="ExternalInput").ap(),
        "bv": nc.dram_tensor("bv", [128, C], F32, kind="ExternalInput").ap(),
        "bp": nc.dram_tensor("bp", [128, C], F32, kind="ExternalInput").ap(),
        "mb": nc.dram_tensor("mb", [128, TC], F32, kind="ExternalInput").ap(),
        "y": nc.dram_tensor("y", [T, C], F32, kind="ExternalOutput").ap(),
    }
    with tile.TileContext(nc) as tc_ctx, ExitStack() as ctx:
        aps["ctx"] = ctx
        _emit_kernel(tc_ctx, aps)
    nc.compile()
    _cache["nc"] = nc
    return nc


def _make_in_maps(inputs):
    bf = ml_dtypes.bfloat16
    x = np.asarray(inputs["x"], np.float32)
    mask = np.asarray(inputs["attn_mask"])
    Wa = np.asarray(inputs["W_attn"], np.float32)
    ba = np.asarray(inputs["b_attn"], np.float32)
    Wp = np.asarray(inputs["W_proj"], np.float32)
    bp = np.asarray(inputs["b_proj"], np.float32)

    # weight layouts: chunk-major, 128-partition-major, contiguous
    wqk = np.ascontiguousarray(
        Wa[:, : 2 * C].reshape(KO, 128, 12, 128).transpose(2, 1, 0, 3)
        .reshape(12, 128, KO * 128)
    ).astype(bf)
    wv = np.ascontiguousarray(
        Wa[:, 2 * C :].reshape(KO, 128, C).transpose(1, 0, 2).reshape(128, KO * C)
    ).astype(bf)
    wp = np.ascontiguousarray(
        Wp.reshape(KO, 128, C).transpose(1, 0, 2).reshape(128, KO * C)
    ).astype(bf)
    bqk = np.ascontiguousarray(ba[: 2 * C].reshape(12, 128).T)
    bv = np.ascontiguousarray(np.broadcast_to(ba[2 * C :], (128, C)))
    bpr = np.ascontiguousarray(np.broadcast_to(bp, (128, C)))
    in_maps = []
    for b in range(B):
        mb = np.where(mask[b] == 0, np.float32(-30.0), np.float32(0.0))
        mb = np.ascontiguousarray(mb.astype(np.float32).reshape(TC, 128).T)
        in_maps.append(
            {
                "xT": np.ascontiguousarray(x[b].T).astype(bf),
                "Wqk": wqk,
                "Wv": wv,
                "Wp": wp,
                "bqk": bqk,
                "bv": bv,
                "bp": bpr,
                "mb": mb,
            }
        )
    return in_maps


def _run(inputs, trace=False):
    nc = _get_program()
    in_maps = _make_in_maps(inputs)
    res = bass_utils.run_bass_kernel_spmd(
        nc, in_maps, core_ids=list(range(B)), trace=trace
    )
    y = np.stack([res.results[b]["y"] for b in range(B)], axis=0)
    return y, res


def kernel(**inputs) -> np.ndarray:
    y, _ = _run(inputs, trace=False)
    return y
